# revision 6
# baseline (speedup 1.0000x reference)
"""Bahdanau-style attention kernel for Trainium2, data-parallel over batch
across 8 NeuronCores.

Reference computation (per batch b):
    e_proj = enc[b] @ We.T            # [S, D]   (We = W[:, 512:], [D, E])
    energy = tanh(e_proj + hidden[b] @ Wh.T + bias)
    scores = energy @ v               # [S]
    attn   = softmax(scores)          # [1, S]

Shapes: B=32, S=2048, E=1024, D=512.  Each core handles 4 batches.

v2 design (per core), aimed at a continuous PE matmul stream:
  - enc is DMA-loaded f32 (HWDGE, sync ring), cast f32->fp16 on VectorE,
    then transposed fp16 via the DMA xbar (HWDGE transpose on the scalar
    ring) so the contraction dim e lands on partitions.  This keeps ALL
    transposes off the PE: the baseline's PE-transpose phases didn't
    register as PE activity for the HAM clock gate, so the PE oscillated
    between 1.2/2.4 GHz (~104us throttled).  With a pure MM stream the
    PE stays at K=8/8.
  - main matmul: psum[d128, s512] += WeT[e128, d128].T @ encT[e128, s512]
    (fp16 weights -> FWL fast weight load, overlapped via the PE's
    background weight buffer).
  - tanh fused with the (h_proj + b) bias via ScalarE activation
    (per-partition bias, d is the partition dim).
  - scores via TensorE matvec with v, software-pipelined one unit behind
    the energy matmuls so the PE never waits on ScalarE's tanh.
  - softmax without a full max pass: exp bias uses the max of the first
    512-block as a stability proxy (softmax is shift-invariant; f32 exp
    easily covers the residual +-few-unit range), so exp/partial sums
    stream per block and only a tiny normalize tail remains per batch.
"""

import numpy as np

B, S, E, D = 32, 2048, 1024, 512
N_CORES = 8
BP = B // N_CORES  # batches per core = 4
SBLK = 512  # s-block (psum free dim)
N_SBLK = S // SBLK  # 4
N_ST = SBLK // 128  # 4 s-subtiles per block
N_EC = E // 128  # 8 e-chunks
N_DP = D // 128  # 4 d-chunks
N_KC = D // 128  # 4 k-chunks (hidden proj contraction)

_CACHE = {}


def _build(debug_dumps=False):
    from contextlib import ExitStack

    import concourse.bass as bass
    import concourse.tile as tile
    from concourse import bacc, mybir
    from concourse.masks import make_identity

    F32 = mybir.dt.float32
    F16 = mybir.dt.float16
    AF = mybir.ActivationFunctionType
    AX = mybir.AxisListType

    nc = bacc.Bacc("TRN2", target_bir_lowering=False, debug=False,
                   num_devices=N_CORES)

    hid_d = nc.dram_tensor("hidden", [BP, D], F32, kind="ExternalInput").ap()
    enc_d = nc.dram_tensor("enc", [BP, S, E], F32, kind="ExternalInput").ap()
    w_d = nc.dram_tensor("W", [D, D + E], F32, kind="ExternalInput").ap()
    b_d = nc.dram_tensor("b", [D], F32, kind="ExternalInput").ap()
    v_d = nc.dram_tensor("v", [D], F32, kind="ExternalInput").ap()
    out_d = nc.dram_tensor("out", [BP, S], F32, kind="ExternalOutput").ap()
    if debug_dumps:
        dbg_scores = nc.dram_tensor(
            "dbg_scores", [BP, S], F32, kind="ExternalOutput").ap()
        dbg_enct = nc.dram_tensor(
            "dbg_enct", [128, N_ST, N_EC, 128], F16, kind="ExternalOutput").ap()

    with tile.TileContext(nc) as tc, ExitStack() as ctx:
        consts = ctx.enter_context(tc.tile_pool(name="consts", bufs=1))
        wpool = ctx.enter_context(tc.tile_pool(name="wpool", bufs=1))
        enc_pool = ctx.enter_context(tc.tile_pool(name="enc", bufs=4))
        enc16_pool = ctx.enter_context(tc.tile_pool(name="enc16", bufs=2))
        enct_pool = ctx.enter_context(tc.tile_pool(name="enct", bufs=4))
        en_pool = ctx.enter_context(tc.tile_pool(name="energy", bufs=2))
        small = ctx.enter_context(tc.tile_pool(name="small", bufs=2))
        ps = ctx.enter_context(tc.tile_pool(name="ps", bufs=5, space="PSUM"))
        ps2 = ctx.enter_context(tc.tile_pool(name="ps2", bufs=3, space="PSUM"))

        identity = consts.tile([128, 128], F32)
        make_identity(nc, identity)

        # ---- load weights & small inputs (scalar ring; sync ring is for enc) ----
        w_sb = wpool.tile([128, N_DP, D + E], F32)
        nc.scalar.dma_start(out=w_sb, in_=w_d.rearrange("(dp p) q -> p dp q", p=128))
        hid_sb = consts.tile([BP, D], F32)
        nc.scalar.dma_start(out=hid_sb, in_=hid_d)
        b_sb4 = consts.tile([N_DP, 128], F32)
        nc.scalar.dma_start(out=b_sb4, in_=b_d.rearrange("(dp q) -> dp q", q=128))
        v_sb4 = consts.tile([N_DP, 128], F32)
        nc.scalar.dma_start(out=v_sb4, in_=v_d.rearrange("(dp q) -> dp q", q=128))

        # preload the exp/tanh activation table early (overlaps with DMAs)
        warm = consts.tile([1, 1], F32)
        nc.vector.memset(warm, 0.0)
        nc.scalar.activation(warm, warm, AF.Tanh)

        # ---- transpose We -> WeT [e, d] (fp16), Wh -> WhT [k, d] ----
        wet_sb = consts.tile([128, N_EC, D], F16)
        for ec in range(N_EC):
            pt = ps.tile([128, 512], F32, tag="pe")
            with tc.tile_critical():
                for dp in range(N_DP):
                    nc.tensor.matmul(
                        pt[:, dp * 128:(dp + 1) * 128],
                        w_sb[:, dp, D + ec * 128: D + (ec + 1) * 128],
                        identity, is_transpose=True,
                        start=(dp == 0), stop=(dp == N_DP - 1),
                    )
            nc.vector.tensor_copy(wet_sb[:, ec, :], pt)

        wht_sb = consts.tile([128, N_KC, D], F32)
        for kc in range(N_KC):
            pt = ps.tile([128, 512], F32, tag="pe")
            with tc.tile_critical():
                for dp in range(N_DP):
                    nc.tensor.matmul(
                        pt[:, dp * 128:(dp + 1) * 128],
                        w_sb[:, dp, kc * 128:(kc + 1) * 128],
                        identity, is_transpose=True,
                        start=(dp == 0), stop=(dp == N_DP - 1),
                    )
            nc.scalar.copy(wht_sb[:, kc, :], pt)

        # ---- hidden^T [k, b] ----
        hidt_sb = consts.tile([128, N_KC, BP], F32)
        for kc in range(N_KC):
            pt = ps2.tile([128, 16], F32, tag="sc")
            nc.tensor.transpose(
                pt[:, 0:BP], hid_sb[:, kc * 128:(kc + 1) * 128],
                identity[0:BP, 0:BP],
            )
            nc.vector.tensor_copy(hidt_sb[:, kc, :], pt[:, 0:BP])

        # ---- b^T, v^T  [128, dp] ----
        bt_sb = consts.tile([128, N_DP], F32)
        pt = ps2.tile([128, 16], F32, tag="sc")
        nc.tensor.transpose(pt[:, 0:N_DP], b_sb4, identity[0:N_DP, 0:N_DP])
        nc.vector.tensor_copy(bt_sb, pt[:, 0:N_DP])

        vt_sb = consts.tile([128, N_DP], F16)
        pt = ps2.tile([128, 16], F32, tag="sc")
        nc.tensor.transpose(pt[:, 0:N_DP], v_sb4, identity[0:N_DP, 0:N_DP])
        nc.vector.tensor_copy(vt_sb, pt[:, 0:N_DP])

        # ---- h_projT + bias -> hbT [128, dp, b] ----
        hbt_sb = consts.tile([128, N_DP, BP], F32)
        for dp in range(N_DP):
            ph = ps2.tile([128, 16], F32, tag="sc")
            for kc in range(N_KC):
                nc.tensor.matmul(
                    ph[:, 0:BP],
                    wht_sb[:, kc, dp * 128:(dp + 1) * 128],
                    hidt_sb[:, kc, :],
                    start=(kc == 0), stop=(kc == N_KC - 1),
                )
            nc.vector.tensor_scalar_add(
                hbt_sb[:, dp, :], ph[:, 0:BP], bt_sb[:, dp:dp + 1]
            )

        # ---- main pipeline over 16 (batch, sblk) units ----
        # Per-batch softmax state
        batch_state = {}

        def emit_scores(bi, sblk, energy):
            """Scores matvec + streamed exp/partial-sum for one unit."""
            psc = ps2.tile([1, SBLK], F32, tag="sc")
            for dp in range(N_DP):
                nc.tensor.matmul(
                    psc, vt_sb[:, dp:dp + 1], energy[:, dp, :],
                    start=(dp == 0), stop=(dp == N_DP - 1),
                )
            if sblk == 0:
                m0 = small.tile([1, 1], F32, tag="m0")
                nc.vector.reduce_max(m0, psc, axis=AX.X)
                negm = small.tile([1, 1], F32, tag="negm")
                nc.vector.tensor_scalar_mul(negm, m0, -1.0)
                prob = small.tile([1, S], F32, tag="prob")
                partials = small.tile([1, N_SBLK], F32, tag="part")
                batch_state[bi] = (negm, prob, partials)
            negm, prob, partials = batch_state[bi]
            if debug_dumps:
                nc.scalar.copy(
                    dbg_scores_sb[:, sblk * SBLK:(sblk + 1) * SBLK], psc)
            nc.scalar.activation(
                prob[:, sblk * SBLK:(sblk + 1) * SBLK], psc, AF.Exp,
                bias=negm, scale=1.0,
            )
            nc.vector.reduce_sum(
                partials[:, sblk:sblk + 1],
                prob[:, sblk * SBLK:(sblk + 1) * SBLK], axis=AX.X)
            if sblk == N_SBLK - 1:
                ssum = small.tile([1, 1], F32, tag="ssum")
                nc.vector.reduce_sum(ssum, partials, axis=AX.X)
                rtot = small.tile([1, 1], F32, tag="rtot")
                nc.vector.reciprocal(rtot, ssum)
                attn = small.tile([1, S], F32, tag="attn")
                nc.vector.tensor_scalar_mul(attn, prob, rtot)
                nc.scalar.dma_start(out=out_d[bi], in_=attn)
                if debug_dumps:
                    nc.sync.dma_start(
                        out=dbg_scores[bi:bi + 1, :], in_=dbg_scores_sb)
                del batch_state[bi]

        if debug_dumps:
            dbg_scores_sb = small.tile([1, S], F32, tag="dbgsc")

        prev = None  # pending (bi, sblk, energy) for the scores pipeline
        for bi in range(BP):
            for sblk in range(N_SBLK):
                # enc load on the SWDGE (gpsimd) ring: keeps both HWDGE
                # rings free for the xbar transposes / ScalarE's tanh
                enc32 = enc_pool.tile([128, N_ST, E], F32, tag="enc32")
                nc.gpsimd.dma_start(
                    out=enc32,
                    in_=enc_d[bi, sblk * SBLK:(sblk + 1) * SBLK, :].rearrange(
                        "(st p) e -> p st e", p=128
                    ),
                )
                enc16 = enc16_pool.tile([128, N_ST, E], F16, tag="enc16")
                nc.vector.tensor_copy(enc16[:, 0:2, :], enc32[:, 0:2, :])
                nc.vector.tensor_copy(enc16[:, 2:4, :], enc32[:, 2:4, :])

                # fp16 transpose on the DMA xbar (sync HWDGE ring — the
                # ~5us/call ucode dispatch must not head-of-line-block
                # ScalarE's tanh stream):
                # out[q, (st, ec), j] = enc16[j, st*1024 + ec*128 + q]
                #   -> enct[q, st, ec, j] = encT[e=ec*128+q, s=st*128+j]
                enct = enct_pool.tile([128, N_ST, N_EC, 128], F16, tag="enct")
                nc.sync.dma_start(
                    out=enct.rearrange("p a b j -> p (a b) j"),
                    in_=enc16.rearrange("p a e -> p (a e)"),
                    transpose=True,
                )
                if debug_dumps and bi == 0 and sblk == 0:
                    nc.sync.dma_start(out=dbg_enct, in_=enct)

                energy = en_pool.tile([128, N_DP, SBLK], F16, tag="energy")
                for dp in range(N_DP):
                    pe = ps.tile([128, SBLK], F32, tag="pe")
                    for ec in range(N_EC):
                        nc.tensor.matmul(
                            pe,
                            wet_sb[:, ec, dp * 128:(dp + 1) * 128],
                            enct[:, :, ec, :],
                            start=(ec == 0), stop=(ec == N_EC - 1),
                        )
                    nc.scalar.activation(
                        energy[:, dp, :], pe, AF.Tanh,
                        bias=hbt_sb[:, dp, bi:bi + 1], scale=1.0,
                    )
                    # scores for the previous unit, mid-stream so the PE
                    # never waits on ScalarE's tanh of THIS unit
                    if dp == 1 and prev is not None:
                        emit_scores(*prev)
                        prev = None
                prev = (bi, sblk, energy)

        emit_scores(*prev)

    nc.compile()
    return nc


def _get_nc():
    if "nc" not in _CACHE:
        _CACHE["nc"] = _build()
    return _CACHE["nc"]


def kernel(hidden, encoder_outputs, W, b, v):
    from concourse.bass_utils import run_bass_kernel_spmd

    nc = _get_nc()
    hidden = np.ascontiguousarray(hidden, dtype=np.float32)
    encoder_outputs = np.ascontiguousarray(encoder_outputs, dtype=np.float32)
    W = np.ascontiguousarray(W, dtype=np.float32)
    b = np.ascontiguousarray(b, dtype=np.float32)
    v = np.ascontiguousarray(v, dtype=np.float32)

    in_maps = [
        {
            "hidden": hidden[c * BP:(c + 1) * BP],
            "enc": encoder_outputs[c * BP:(c + 1) * BP],
            "W": W,
            "b": b,
            "v": v,
        }
        for c in range(N_CORES)
    ]
    r = run_bass_kernel_spmd(nc, in_maps, list(range(N_CORES)))
    out = np.concatenate([r.results[c]["out"] for c in range(N_CORES)], axis=0)
    return out[:, None, :].astype(np.float32)


# revision 9
# speedup vs baseline: 1.0156x; 1.0156x over previous
"""Bahdanau-style attention kernel for Trainium2, data-parallel over batch
across 8 NeuronCores.

Reference computation (per batch b):
    e_proj = enc[b] @ We.T            # [S, D]   (We = W[:, 512:], [D, E])
    energy = tanh(e_proj + hidden[b] @ Wh.T + bias)
    scores = energy @ v               # [S]
    attn   = softmax(scores)          # [1, S]

Shapes: B=32, S=2048, E=1024, D=512.  Each core handles 4 batches.

v2 design (per core), aimed at a continuous PE matmul stream:
  - enc is DMA-loaded f32 (HWDGE, sync ring), cast f32->fp16 on VectorE,
    then transposed fp16 via the DMA xbar (HWDGE transpose on the scalar
    ring) so the contraction dim e lands on partitions.  This keeps ALL
    transposes off the PE: the baseline's PE-transpose phases didn't
    register as PE activity for the HAM clock gate, so the PE oscillated
    between 1.2/2.4 GHz (~104us throttled).  With a pure MM stream the
    PE stays at K=8/8.
  - main matmul: psum[d128, s512] += WeT[e128, d128].T @ encT[e128, s512]
    (fp16 weights -> FWL fast weight load, overlapped via the PE's
    background weight buffer).
  - tanh fused with the (h_proj + b) bias via ScalarE activation
    (per-partition bias, d is the partition dim).
  - scores via TensorE matvec with v, software-pipelined one unit behind
    the energy matmuls so the PE never waits on ScalarE's tanh.
  - softmax without a full max pass: exp bias uses the max of the first
    512-block as a stability proxy (softmax is shift-invariant; f32 exp
    easily covers the residual +-few-unit range), so exp/partial sums
    stream per block and only a tiny normalize tail remains per batch.
"""

import numpy as np

B, S, E, D = 32, 2048, 1024, 512
N_CORES = 8
BP = B // N_CORES  # batches per core = 4
SBLK = 512  # s-block (psum free dim)
N_SBLK = S // SBLK  # 4
N_ST = SBLK // 128  # 4 s-subtiles per block
N_EC = E // 128  # 8 e-chunks
N_DP = D // 128  # 4 d-chunks
N_KC = D // 128  # 4 k-chunks (hidden proj contraction)

_CACHE = {}


def _build(debug_dumps=False):
    from contextlib import ExitStack

    import concourse.bass as bass
    import concourse.tile as tile
    from concourse import bacc, mybir
    from concourse.masks import make_identity

    F32 = mybir.dt.float32
    F16 = mybir.dt.float16
    AF = mybir.ActivationFunctionType
    AX = mybir.AxisListType

    nc = bacc.Bacc("TRN2", target_bir_lowering=False, debug=False,
                   num_devices=N_CORES)

    hid_d = nc.dram_tensor("hidden", [BP, D], F32, kind="ExternalInput").ap()
    enc_d = nc.dram_tensor("enc", [BP, S, E], F32, kind="ExternalInput").ap()
    w_d = nc.dram_tensor("W", [D, D + E], F32, kind="ExternalInput").ap()
    b_d = nc.dram_tensor("b", [D], F32, kind="ExternalInput").ap()
    v_d = nc.dram_tensor("v", [D], F32, kind="ExternalInput").ap()
    out_d = nc.dram_tensor("out", [BP, S], F32, kind="ExternalOutput").ap()
    if debug_dumps:
        dbg_scores = nc.dram_tensor(
            "dbg_scores", [BP, S], F32, kind="ExternalOutput").ap()
        dbg_enct = nc.dram_tensor(
            "dbg_enct", [128, N_ST, N_EC, 128], F16, kind="ExternalOutput").ap()

    with tile.TileContext(nc) as tc, ExitStack() as ctx:
        consts = ctx.enter_context(tc.tile_pool(name="consts", bufs=1))
        wpool = ctx.enter_context(tc.tile_pool(name="wpool", bufs=1))
        enc_pool = ctx.enter_context(tc.tile_pool(name="enc", bufs=4))
        enc16_pool = ctx.enter_context(tc.tile_pool(name="enc16", bufs=2))
        enct_pool = ctx.enter_context(tc.tile_pool(name="enct", bufs=4))
        en_pool = ctx.enter_context(tc.tile_pool(name="energy", bufs=2))
        small = ctx.enter_context(tc.tile_pool(name="small", bufs=2))
        ps = ctx.enter_context(tc.tile_pool(name="ps", bufs=5, space="PSUM"))
        ps2 = ctx.enter_context(tc.tile_pool(name="ps2", bufs=3, space="PSUM"))

        identity = consts.tile([128, 128], F32)
        make_identity(nc, identity)

        # ---- per-unit producer helpers ----
        # Engine-role discipline (the point of this structure): the DVE
        # queue carries ONLY the f32->f16 casts.  Any DVE op that
        # transitively depends on PE/ScalarE work would head-of-line-block
        # a later cast (strict FIFO), closing a PE->Scalar->DVE->xbar->PE
        # loop that paces the whole kernel (measured 16us/unit in v3).
        def emit_load(bi, sblk, ring):
            enc32 = enc_pool.tile([128, N_ST, E], F32, tag="enc32",
                                  name=f"enc32_{bi}_{sblk}")
            ring.dma_start(
                out=enc32,
                in_=enc_d[bi, sblk * SBLK:(sblk + 1) * SBLK, :].rearrange(
                    "(st p) e -> p st e", p=128
                ),
            )
            return enc32

        def emit_cast(enc32, bi, sblk):
            enc16 = enc16_pool.tile([128, N_ST, E], F16, tag="enc16",
                                    name=f"enc16_{bi}_{sblk}")
            nc.vector.tensor_copy(enc16[:, 0:2, :], enc32[:, 0:2, :])
            nc.vector.tensor_copy(enc16[:, 2:4, :], enc32[:, 2:4, :])
            return enc16

        def emit_xbar(enc16, bi, sblk):
            # fp16 transpose on the DMA xbar (sync HWDGE ring):
            # out[q, (st, ec), j] = enc16[j, st*1024 + ec*128 + q]
            #   -> enct[q, st, ec, j] = encT[e=ec*128+q, s=st*128+j]
            enct = enct_pool.tile([128, N_ST, N_EC, 128], F16, tag="enct",
                                  name=f"enct_{bi}_{sblk}")
            nc.sync.dma_start(
                out=enct.rearrange("p a b j -> p (a b) j"),
                in_=enc16.rearrange("p a e -> p (a e)"),
                transpose=True,
            )
            return enct

        # ---- load weights & small inputs ----
        # W on the sync ring FIRST (it gates the PE setup transposes);
        # then prefetch units 0/1 so their casts lead the DVE queue and
        # their xbars lead the sync ring.
        w_sb = wpool.tile([128, N_DP, D + E], F32)
        nc.sync.dma_start(out=w_sb, in_=w_d.rearrange("(dp p) q -> p dp q", p=128))

        prefetched = {}
        for pu, ring in ((0, nc.scalar), (1, nc.gpsimd)):
            enc32 = emit_load(0, pu, ring)
            enc16 = emit_cast(enc32, 0, pu)
            prefetched[pu] = emit_xbar(enc16, 0, pu)
        hid_sb = consts.tile([BP, D], F32)
        nc.scalar.dma_start(out=hid_sb, in_=hid_d)
        b_sb4 = consts.tile([N_DP, 128], F32)
        nc.scalar.dma_start(out=b_sb4, in_=b_d.rearrange("(dp q) -> dp q", q=128))
        v_sb4 = consts.tile([N_DP, 128], F32)
        nc.scalar.dma_start(out=v_sb4, in_=v_d.rearrange("(dp q) -> dp q", q=128))

        # preload the exp/tanh activation table early (overlaps with DMAs)
        warm = consts.tile([1, 1], F32)
        nc.vector.memset(warm, 0.0)
        nc.scalar.activation(warm, warm, AF.Tanh)

        # ---- transpose We -> WeT [e, d] (fp16), Wh -> WhT [k, d] ----
        wet_sb = consts.tile([128, N_EC, D], F16)
        for ec in range(N_EC):
            pt = ps.tile([128, 512], F32, tag="pe")
            with tc.tile_critical():
                for dp in range(N_DP):
                    nc.tensor.matmul(
                        pt[:, dp * 128:(dp + 1) * 128],
                        w_sb[:, dp, D + ec * 128: D + (ec + 1) * 128],
                        identity, is_transpose=True,
                        start=(dp == 0), stop=(dp == N_DP - 1),
                    )
            nc.vector.tensor_copy(wet_sb[:, ec, :], pt)

        wht_sb = consts.tile([128, N_KC, D], F32)
        for kc in range(N_KC):
            pt = ps.tile([128, 512], F32, tag="pe")
            with tc.tile_critical():
                for dp in range(N_DP):
                    nc.tensor.matmul(
                        pt[:, dp * 128:(dp + 1) * 128],
                        w_sb[:, dp, kc * 128:(kc + 1) * 128],
                        identity, is_transpose=True,
                        start=(dp == 0), stop=(dp == N_DP - 1),
                    )
            nc.scalar.copy(wht_sb[:, kc, :], pt)

        # ---- hidden^T [k, b] ----
        hidt_sb = consts.tile([128, N_KC, BP], F32)
        for kc in range(N_KC):
            pt = ps2.tile([128, 16], F32, tag="sc")
            nc.tensor.transpose(
                pt[:, 0:BP], hid_sb[:, kc * 128:(kc + 1) * 128],
                identity[0:BP, 0:BP],
            )
            nc.vector.tensor_copy(hidt_sb[:, kc, :], pt[:, 0:BP])

        # ---- b^T, v^T  [128, dp] ----
        bt_sb = consts.tile([128, N_DP], F32)
        pt = ps2.tile([128, 16], F32, tag="sc")
        nc.tensor.transpose(pt[:, 0:N_DP], b_sb4, identity[0:N_DP, 0:N_DP])
        nc.vector.tensor_copy(bt_sb, pt[:, 0:N_DP])

        vt_sb = consts.tile([128, N_DP], F16)
        pt = ps2.tile([128, 16], F32, tag="sc")
        nc.tensor.transpose(pt[:, 0:N_DP], v_sb4, identity[0:N_DP, 0:N_DP])
        nc.vector.tensor_copy(vt_sb, pt[:, 0:N_DP])

        # ---- h_projT + bias -> hbT [128, dp, b] ----
        hbt_sb = consts.tile([128, N_DP, BP], F32)
        for dp in range(N_DP):
            ph = ps2.tile([128, 16], F32, tag="sc")
            for kc in range(N_KC):
                nc.tensor.matmul(
                    ph[:, 0:BP],
                    wht_sb[:, kc, dp * 128:(dp + 1) * 128],
                    hidt_sb[:, kc, :],
                    start=(kc == 0), stop=(kc == N_KC - 1),
                )
            nc.vector.tensor_scalar_add(
                hbt_sb[:, dp, :], ph[:, 0:BP], bt_sb[:, dp:dp + 1]
            )

        # ---- main pipeline over 16 (batch, sblk) units ----
        # Per-batch softmax state
        batch_state = {}

        def emit_scores(bi, sblk, energy):
            """Scores matvec + streamed exp with fused partial sums."""
            psc = ps2.tile([1, SBLK], F32, tag="sc")
            for dp in range(N_DP):
                nc.tensor.matmul(
                    psc, vt_sb[:, dp:dp + 1], energy[:, dp, :],
                    start=(dp == 0), stop=(dp == N_DP - 1),
                )
            if sblk == 0:
                prob = small.tile([1, S], F32, tag="prob")
                partials = small.tile([1, N_SBLK], F32, tag="part")
                batch_state[bi] = (prob, partials)
            prob, partials = batch_state[bi]
            if debug_dumps:
                nc.scalar.copy(
                    dbg_scores_sb[:, sblk * SBLK:(sblk + 1) * SBLK], psc)
            # exp without max-subtraction: softmax is shift-invariant and
            # |scores| <~ 40 stays well inside the f32 exp range; accum_out
            # yields the block sum for free (keeps reductions off the DVE).
            nc.scalar.activation(
                prob[:, sblk * SBLK:(sblk + 1) * SBLK], psc, AF.Exp,
                bias=0.0, scale=1.0,
                accum_out=partials[:, sblk:sblk + 1],
            )
            if sblk == N_SBLK - 1:
                ssum = small.tile([1, 1], F32, tag="ssum")
                nc.gpsimd.reduce_sum(ssum, partials, axis=AX.XYZWC)
                rtot = small.tile([1, 1], F32, tag="rtot")
                nc.vector.reciprocal(rtot, ssum)
                attn = small.tile([1, S], F32, tag="attn")
                nc.scalar.activation(attn, prob, AF.Copy, scale=rtot)
                nc.scalar.dma_start(out=out_d[bi], in_=attn)
                if debug_dumps:
                    nc.sync.dma_start(
                        out=dbg_scores[bi:bi + 1, :], in_=dbg_scores_sb)
                del batch_state[bi]

        if debug_dumps:
            dbg_scores_sb = small.tile([1, S], F32, tag="dbgsc")

        prev = None  # pending (bi, sblk, energy) for the scores pipeline
        for bi in range(BP):
            for sblk in range(N_SBLK):
                u = bi * N_SBLK + sblk
                if u in prefetched:
                    enct = prefetched.pop(u)
                else:
                    enc32 = emit_load(bi, sblk, nc.gpsimd)
                    enc16 = emit_cast(enc32, bi, sblk)
                    enct = emit_xbar(enc16, bi, sblk)
                if debug_dumps and bi == 0 and sblk == 0:
                    nc.sync.dma_start(out=dbg_enct, in_=enct)

                energy = en_pool.tile([128, N_DP, SBLK], F16, tag="energy")
                for dp in range(N_DP):
                    pe = ps.tile([128, SBLK], F32, tag="pe")
                    for ec in range(N_EC):
                        nc.tensor.matmul(
                            pe,
                            wet_sb[:, ec, dp * 128:(dp + 1) * 128],
                            enct[:, :, ec, :],
                            start=(ec == 0), stop=(ec == N_EC - 1),
                        )
                    nc.scalar.activation(
                        energy[:, dp, :], pe, AF.Tanh,
                        bias=hbt_sb[:, dp, bi:bi + 1], scale=1.0,
                    )
                    # scores for the previous unit, mid-stream so the PE
                    # never waits on ScalarE's tanh of THIS unit
                    if dp == 1 and prev is not None:
                        emit_scores(*prev)
                        prev = None
                prev = (bi, sblk, energy)

        emit_scores(*prev)

    nc.compile()
    return nc


def _get_nc():
    if "nc" not in _CACHE:
        _CACHE["nc"] = _build()
    return _CACHE["nc"]


def kernel(hidden, encoder_outputs, W, b, v):
    from concourse.bass_utils import run_bass_kernel_spmd

    nc = _get_nc()
    hidden = np.ascontiguousarray(hidden, dtype=np.float32)
    encoder_outputs = np.ascontiguousarray(encoder_outputs, dtype=np.float32)
    W = np.ascontiguousarray(W, dtype=np.float32)
    b = np.ascontiguousarray(b, dtype=np.float32)
    v = np.ascontiguousarray(v, dtype=np.float32)

    in_maps = [
        {
            "hidden": hidden[c * BP:(c + 1) * BP],
            "enc": encoder_outputs[c * BP:(c + 1) * BP],
            "W": W,
            "b": b,
            "v": v,
        }
        for c in range(N_CORES)
    ]
    r = run_bass_kernel_spmd(nc, in_maps, list(range(N_CORES)))
    out = np.concatenate([r.results[c]["out"] for c in range(N_CORES)], axis=0)
    return out[:, None, :].astype(np.float32)


# revision 13
# speedup vs baseline: 1.0595x; 1.0432x over previous
"""Bahdanau-style attention kernel for Trainium2, data-parallel over batch
across 8 NeuronCores.

Reference computation (per batch b):
    e_proj = enc[b] @ We.T            # [S, D]   (We = W[:, 512:], [D, E])
    energy = tanh(e_proj + hidden[b] @ Wh.T + bias)
    scores = energy @ v               # [S]
    attn   = softmax(scores)          # [1, S]

Shapes: B=32, S=2048, E=1024, D=512.  Each core handles 4 batches.

v2 design (per core), aimed at a continuous PE matmul stream:
  - enc is DMA-loaded f32 (HWDGE, sync ring), cast f32->fp16 on VectorE,
    then transposed fp16 via the DMA xbar (HWDGE transpose on the scalar
    ring) so the contraction dim e lands on partitions.  This keeps ALL
    transposes off the PE: the baseline's PE-transpose phases didn't
    register as PE activity for the HAM clock gate, so the PE oscillated
    between 1.2/2.4 GHz (~104us throttled).  With a pure MM stream the
    PE stays at K=8/8.
  - main matmul: psum[d128, s512] += WeT[e128, d128].T @ encT[e128, s512]
    (fp16 weights -> FWL fast weight load, overlapped via the PE's
    background weight buffer).
  - tanh fused with the (h_proj + b) bias via ScalarE activation
    (per-partition bias, d is the partition dim).
  - scores via TensorE matvec with v, software-pipelined one unit behind
    the energy matmuls so the PE never waits on ScalarE's tanh.
  - softmax without a full max pass: exp bias uses the max of the first
    512-block as a stability proxy (softmax is shift-invariant; f32 exp
    easily covers the residual +-few-unit range), so exp/partial sums
    stream per block and only a tiny normalize tail remains per batch.
"""

import numpy as np

B, S, E, D = 32, 2048, 1024, 512
N_CORES = 8
BP = B // N_CORES  # batches per core = 4
SBLK = 512  # s-block (psum free dim)
N_SBLK = S // SBLK  # 4
N_ST = SBLK // 128  # 4 s-subtiles per block
N_EC = E // 128  # 8 e-chunks
N_DP = D // 128  # 4 d-chunks
N_KC = D // 128  # 4 k-chunks (hidden proj contraction)

_CACHE = {}


def _build(debug_dumps=False):
    from contextlib import ExitStack

    import concourse.bass as bass
    import concourse.tile as tile
    from concourse import bacc, mybir
    from concourse.masks import make_identity

    F32 = mybir.dt.float32
    F16 = mybir.dt.float16
    AF = mybir.ActivationFunctionType
    AX = mybir.AxisListType

    nc = bacc.Bacc("TRN2", target_bir_lowering=False, debug=False,
                   num_devices=N_CORES)

    hid_d = nc.dram_tensor("hidden", [BP, D], F32, kind="ExternalInput").ap()
    enc_d = nc.dram_tensor("enc", [BP, S, E], F32, kind="ExternalInput").ap()
    w_d = nc.dram_tensor("W", [D, D + E], F32, kind="ExternalInput").ap()
    b_d = nc.dram_tensor("b", [D], F32, kind="ExternalInput").ap()
    v_d = nc.dram_tensor("v", [D], F32, kind="ExternalInput").ap()
    out_d = nc.dram_tensor("out", [BP, S], F32, kind="ExternalOutput").ap()
    if debug_dumps:
        dbg_scores = nc.dram_tensor(
            "dbg_scores", [BP, S], F32, kind="ExternalOutput").ap()
        dbg_enct = nc.dram_tensor(
            "dbg_enct", [128, N_ST, N_EC, 128], F16, kind="ExternalOutput").ap()

    with tile.TileContext(nc) as tc, ExitStack() as ctx:
        consts = ctx.enter_context(tc.tile_pool(name="consts", bufs=1))
        wpool = ctx.enter_context(tc.tile_pool(name="wpool", bufs=1))
        enc_pool = ctx.enter_context(tc.tile_pool(name="enc", bufs=3))
        enc16_pool = ctx.enter_context(tc.tile_pool(name="enc16", bufs=3))
        enct_pool = ctx.enter_context(tc.tile_pool(name="enct", bufs=4))
        en_pool = ctx.enter_context(tc.tile_pool(name="energy", bufs=2))
        small = ctx.enter_context(tc.tile_pool(name="small", bufs=2))
        ps = ctx.enter_context(tc.tile_pool(name="ps", bufs=6, space="PSUM"))
        ps2 = ctx.enter_context(tc.tile_pool(name="ps2", bufs=2, space="PSUM"))

        # ---- per-unit producer helpers ----
        # Engine-role discipline (the point of this structure): the DVE
        # queue carries ONLY the f32->f16 casts.  Any DVE op that
        # transitively depends on PE/ScalarE work would head-of-line-block
        # a later cast (strict FIFO), closing a PE->Scalar->DVE->xbar->PE
        # loop that paces the whole kernel (measured 16us/unit in v3).
        def emit_load(bi, sblk, ring):
            enc32 = enc_pool.tile([128, N_ST, E], F32, tag="enc32",
                                  name=f"enc32_{bi}_{sblk}")
            ring.dma_start(
                out=enc32,
                in_=enc_d[bi, sblk * SBLK:(sblk + 1) * SBLK, :].rearrange(
                    "(st p) e -> p st e", p=128
                ),
            )
            return enc32

        def emit_cast(enc32, bi, sblk):
            enc16 = enc16_pool.tile([128, N_ST, E], F16, tag="enc16",
                                    name=f"enc16_{bi}_{sblk}")
            nc.vector.tensor_copy(enc16[:, 0:2, :], enc32[:, 0:2, :])
            nc.vector.tensor_copy(enc16[:, 2:4, :], enc32[:, 2:4, :])
            return enc16

        def emit_xbar(enc16, bi, sblk):
            # fp16 transpose on the DMA xbar (sync HWDGE ring):
            # out[q, (st, ec), j] = enc16[j, st*1024 + ec*128 + q]
            #   -> enct[q, st, ec, j] = encT[e=ec*128+q, s=st*128+j]
            enct = enct_pool.tile([128, N_ST, N_EC, 128], F16, tag="enct",
                                  name=f"enct_{bi}_{sblk}")
            nc.sync.dma_start(
                out=enct.rearrange("p a b j -> p (a b) j"),
                in_=enc16.rearrange("p a e -> p (a e)"),
                transpose=True,
            )
            return enct

        # ---- load weights & small inputs ----
        # Zero PE-transpose setup: W goes through the same cast+xbar path
        # as enc (the v4 PE-transpose setup serialized ~66us of fp32
        # two-pass transposes ping-ponging with DVE copies before the
        # first energy matmul could start).  hidden/b/v are loaded
        # pre-transposed straight from DRAM via rearranged APs (SWDGE
        # casts the f16 ones during the DMA).
        w_sb = wpool.tile([128, N_DP, D + E], F32)
        nc.sync.dma_start(out=w_sb, in_=w_d.rearrange("(dp p) q -> p dp q", p=128))

        # prefetch unit 0 so its cast leads the DVE queue and its xbar
        # leads the sync ring; unit 1 right behind on the gpsimd ring.
        prefetched = {}
        for pu, ring in ((0, nc.scalar), (1, nc.gpsimd)):
            enc32 = emit_load(0, pu, ring)
            enc16 = emit_cast(enc32, 0, pu)
            prefetched[pu] = emit_xbar(enc16, 0, pu)

        # W f32 -> f16 on DVE, then one xbar call transposes ALL of W:
        #   wt[q, dp, c, j] = W[dp*128+j, c*128+q]
        # c in [0,4) is Wh (contraction k), c in [4,12) is We (contraction e).
        NWC = (D + E) // 128  # 12 column-chunks of W
        w16 = wpool.tile([128, N_DP, D + E], F16)
        nc.vector.tensor_copy(w16[:, 0:2, :], w_sb[:, 0:2, :])
        nc.vector.tensor_copy(w16[:, 2:4, :], w_sb[:, 2:4, :])
        wt_sb = consts.tile([128, N_DP, NWC, 128], F16)
        nc.sync.dma_start(
            out=wt_sb.rearrange("p a b j -> p (a b) j"),
            in_=w16.rearrange("p a q -> p (a q)"),
            transpose=True,
        )

        # hidden^T [k, b] in f16 (cast-DMA), b^T f32, v^T f16 (cast-DMA)
        hidt_sb = consts.tile([128, N_KC, BP], F16)
        for b in range(BP):
            nc.gpsimd.dma_start(
                out=hidt_sb[:, :, b],
                in_=hid_d[b].rearrange("(kc p) -> p kc", p=128))
        bt_sb = consts.tile([128, N_DP], F32)
        nc.scalar.dma_start(
            out=bt_sb, in_=b_d.rearrange("(dp p) -> p dp", p=128))
        vt_sb = consts.tile([128, N_DP], F16)
        nc.gpsimd.dma_start(
            out=vt_sb, in_=v_d.rearrange("(dp p) -> p dp", p=128))

        # preload the exp/tanh activation table early (overlaps with DMAs)
        warm = consts.tile([1, 1], F32)
        nc.vector.memset(warm, 0.0)
        nc.scalar.activation(warm, warm, AF.Tanh)

        # ---- h_projT + bias -> hbT [128, dp, b] ----
        hbt_sb = consts.tile([128, N_DP, BP], F32)
        for dp in range(N_DP):
            ph = ps2.tile([128, 16], F32, tag="sc")
            for kc in range(N_KC):
                nc.tensor.matmul(
                    ph[:, 0:BP],
                    wt_sb[:, dp, kc, :],
                    hidt_sb[:, kc, :],
                    start=(kc == 0), stop=(kc == N_KC - 1),
                )
            nc.vector.tensor_scalar_add(
                hbt_sb[:, dp, :], ph[:, 0:BP], bt_sb[:, dp:dp + 1]
            )

        # ---- main pipeline over 16 (batch, sblk) units ----
        # Per-batch softmax state
        batch_state = {}

        def emit_scores(bi, sblk, energy):
            """Scores matvec + streamed exp with fused partial sums."""
            psc = ps2.tile([1, SBLK], F32, tag="sc")
            for dp in range(N_DP):
                nc.tensor.matmul(
                    psc, vt_sb[:, dp:dp + 1], energy[:, dp, :],
                    start=(dp == 0), stop=(dp == N_DP - 1),
                )
            if sblk == 0:
                prob = small.tile([1, S], F32, tag="prob")
                partials = small.tile([1, N_SBLK], F32, tag="part")
                batch_state[bi] = (prob, partials)
            prob, partials = batch_state[bi]
            if debug_dumps:
                nc.scalar.copy(
                    dbg_scores_sb[:, sblk * SBLK:(sblk + 1) * SBLK], psc)
            # exp without max-subtraction: softmax is shift-invariant and
            # |scores| <~ 40 stays well inside the f32 exp range; accum_out
            # yields the block sum for free (keeps reductions off the DVE).
            nc.scalar.activation(
                prob[:, sblk * SBLK:(sblk + 1) * SBLK], psc, AF.Exp,
                bias=0.0, scale=1.0,
                accum_out=partials[:, sblk:sblk + 1],
            )
            if sblk == N_SBLK - 1:
                ssum = small.tile([1, 1], F32, tag="ssum")
                nc.gpsimd.reduce_sum(ssum, partials, axis=AX.XYZWC)
                rtot = small.tile([1, 1], F32, tag="rtot")
                nc.vector.reciprocal(rtot, ssum)
                attn = small.tile([1, S], F32, tag="attn")
                nc.scalar.activation(attn, prob, AF.Copy, scale=rtot)
                nc.scalar.dma_start(out=out_d[bi], in_=attn)
                if debug_dumps:
                    nc.sync.dma_start(
                        out=dbg_scores[bi:bi + 1, :], in_=dbg_scores_sb)
                del batch_state[bi]

        if debug_dumps:
            dbg_scores_sb = small.tile([1, S], F32, tag="dbgsc")

        prev = None  # pending (bi, sblk, energy) for the scores pipeline
        for bi in range(BP):
            for sblk in range(N_SBLK):
                u = bi * N_SBLK + sblk
                if u in prefetched:
                    enct = prefetched.pop(u)
                else:
                    enc32 = emit_load(bi, sblk, nc.gpsimd)
                    enc16 = emit_cast(enc32, bi, sblk)
                    enct = emit_xbar(enc16, bi, sblk)
                if debug_dumps and bi == 0 and sblk == 0:
                    nc.sync.dma_start(out=dbg_enct, in_=enct)

                energy = en_pool.tile([128, N_DP, SBLK], F16, tag="energy")
                for dp in range(N_DP):
                    pe = ps.tile([128, SBLK], F32, tag="pe")
                    for ec in range(N_EC):
                        nc.tensor.matmul(
                            pe,
                            wt_sb[:, dp, N_KC + ec, :],
                            enct[:, :, ec, :],
                            start=(ec == 0), stop=(ec == N_EC - 1),
                        )
                    nc.scalar.activation(
                        energy[:, dp, :], pe, AF.Tanh,
                        bias=hbt_sb[:, dp, bi:bi + 1], scale=1.0,
                    )
                    # scores for the previous unit, mid-stream so the PE
                    # never waits on ScalarE's tanh of THIS unit
                    if dp == 1 and prev is not None:
                        emit_scores(*prev)
                        prev = None
                prev = (bi, sblk, energy)

        emit_scores(*prev)

    nc.compile()
    return nc


def _get_nc():
    if "nc" not in _CACHE:
        _CACHE["nc"] = _build()
    return _CACHE["nc"]


def kernel(hidden, encoder_outputs, W, b, v):
    from concourse.bass_utils import run_bass_kernel_spmd

    nc = _get_nc()
    hidden = np.ascontiguousarray(hidden, dtype=np.float32)
    encoder_outputs = np.ascontiguousarray(encoder_outputs, dtype=np.float32)
    W = np.ascontiguousarray(W, dtype=np.float32)
    b = np.ascontiguousarray(b, dtype=np.float32)
    v = np.ascontiguousarray(v, dtype=np.float32)

    in_maps = [
        {
            "hidden": hidden[c * BP:(c + 1) * BP],
            "enc": encoder_outputs[c * BP:(c + 1) * BP],
            "W": W,
            "b": b,
            "v": v,
        }
        for c in range(N_CORES)
    ]
    r = run_bass_kernel_spmd(nc, in_maps, list(range(N_CORES)))
    out = np.concatenate([r.results[c]["out"] for c in range(N_CORES)], axis=0)
    return out[:, None, :].astype(np.float32)


# revision 17
# speedup vs baseline: 1.0981x; 1.0364x over previous
"""Bahdanau-style attention kernel for Trainium2, data-parallel over batch
across 8 NeuronCores.

Reference computation (per batch b):
    e_proj = enc[b] @ We.T            # [S, D]   (We = W[:, 512:], [D, E])
    energy = tanh(e_proj + hidden[b] @ Wh.T + bias)
    scores = energy @ v               # [S]
    attn   = softmax(scores)          # [1, S]

Shapes: B=32, S=2048, E=1024, D=512.  Each core handles 4 batches.

v2 design (per core), aimed at a continuous PE matmul stream:
  - enc is DMA-loaded f32 (HWDGE, sync ring), cast f32->fp16 on VectorE,
    then transposed fp16 via the DMA xbar (HWDGE transpose on the scalar
    ring) so the contraction dim e lands on partitions.  This keeps ALL
    transposes off the PE: the baseline's PE-transpose phases didn't
    register as PE activity for the HAM clock gate, so the PE oscillated
    between 1.2/2.4 GHz (~104us throttled).  With a pure MM stream the
    PE stays at K=8/8.
  - main matmul: psum[d128, s512] += WeT[e128, d128].T @ encT[e128, s512]
    (fp16 weights -> FWL fast weight load, overlapped via the PE's
    background weight buffer).
  - tanh fused with the (h_proj + b) bias via ScalarE activation
    (per-partition bias, d is the partition dim).
  - scores via TensorE matvec with v, software-pipelined one unit behind
    the energy matmuls so the PE never waits on ScalarE's tanh.
  - softmax without a full max pass: exp bias uses the max of the first
    512-block as a stability proxy (softmax is shift-invariant; f32 exp
    easily covers the residual +-few-unit range), so exp/partial sums
    stream per block and only a tiny normalize tail remains per batch.
"""

import numpy as np

B, S, E, D = 32, 2048, 1024, 512
N_CORES = 8
BP = B // N_CORES  # batches per core = 4
SBLK = 512  # s-block (psum free dim)
N_SBLK = S // SBLK  # 4
N_ST = SBLK // 128  # 4 s-subtiles per block
N_EC = E // 128  # 8 e-chunks
N_DP = D // 128  # 4 d-chunks
N_KC = D // 128  # 4 k-chunks (hidden proj contraction)

_CACHE = {}


NWC = (D + E) // 128  # 12 column-chunks of W


def _build(debug_dumps=False):
    from contextlib import ExitStack

    import concourse.tile as tile
    from concourse import bacc, mybir

    F32 = mybir.dt.float32
    F16 = mybir.dt.float16
    AF = mybir.ActivationFunctionType
    AX = mybir.AxisListType

    nc = bacc.Bacc("TRN2", target_bir_lowering=False, debug=False,
                   num_devices=N_CORES)

    # Weights/small inputs arrive pre-transposed + pre-cast from the host
    # (pure layout transforms; the device does all the FLOPs):
    #   wt[q, dp, c, j]  = W[dp*128+j, c*128+q]   (c<4 -> Wh, c>=4 -> We)
    #   hidt[p, kc, b]   = hidden[b, kc*128+p]
    #   bt[p, dp]        = b[dp*128+p]
    #   vt[p, dp]        = v[dp*128+p]
    wt_d = nc.dram_tensor("wt", [128, N_DP, NWC, 128], F16,
                          kind="ExternalInput").ap()
    hidt_d = nc.dram_tensor("hidt", [128, N_KC, BP], F16,
                            kind="ExternalInput").ap()
    bt_d = nc.dram_tensor("bt", [128, N_DP], F32, kind="ExternalInput").ap()
    vt_d = nc.dram_tensor("vt", [128, N_DP], F16, kind="ExternalInput").ap()
    enc_d = nc.dram_tensor("enc", [BP, S, E], F32, kind="ExternalInput").ap()
    out_d = nc.dram_tensor("out", [BP, S], F32, kind="ExternalOutput").ap()
    if debug_dumps:
        dbg_scores = nc.dram_tensor(
            "dbg_scores", [BP, S], F32, kind="ExternalOutput").ap()
        dbg_enct = nc.dram_tensor(
            "dbg_enct", [128, N_ST, N_EC, 128], F16, kind="ExternalOutput").ap()

    with tile.TileContext(nc) as tc, ExitStack() as ctx:
        consts = ctx.enter_context(tc.tile_pool(name="consts", bufs=1))
        enc_pool = ctx.enter_context(tc.tile_pool(name="enc", bufs=4))
        enc16_pool = ctx.enter_context(tc.tile_pool(name="enc16", bufs=3))
        enct_pool = ctx.enter_context(tc.tile_pool(name="enct", bufs=5))
        en_pool = ctx.enter_context(tc.tile_pool(name="energy", bufs=2))
        small = ctx.enter_context(tc.tile_pool(name="small", bufs=2))
        ps = ctx.enter_context(tc.tile_pool(name="ps", bufs=6, space="PSUM"))
        ps2 = ctx.enter_context(tc.tile_pool(name="ps2", bufs=2, space="PSUM"))

        # ---- per-unit producer helpers ----
        # Engine-role discipline (the point of this structure): the DVE
        # queue carries ONLY the f32->f16 casts.  Any DVE op that
        # transitively depends on PE/ScalarE work would head-of-line-block
        # a later cast (strict FIFO), closing a PE->Scalar->DVE->xbar->PE
        # loop that paces the whole kernel (measured 16us/unit in v3).
        def emit_load(bi, sblk, ring):
            enc32 = enc_pool.tile([128, N_ST, E], F32, tag="enc32",
                                  name=f"enc32_{bi}_{sblk}")
            ring.dma_start(
                out=enc32,
                in_=enc_d[bi, sblk * SBLK:(sblk + 1) * SBLK, :].rearrange(
                    "(st p) e -> p st e", p=128
                ),
            )
            return enc32

        def emit_cast(enc32, bi, sblk):
            enc16 = enc16_pool.tile([128, N_ST, E], F16, tag="enc16",
                                    name=f"enc16_{bi}_{sblk}")
            nc.vector.tensor_copy(enc16[:, 0:2, :], enc32[:, 0:2, :])
            nc.vector.tensor_copy(enc16[:, 2:4, :], enc32[:, 2:4, :])
            return enc16

        def emit_xbar(enc16, bi, sblk):
            # fp16 transpose on the DMA xbar (sync HWDGE ring):
            # out[q, (st, ec), j] = enc16[j, st*1024 + ec*128 + q]
            #   -> enct[q, st, ec, j] = encT[e=ec*128+q, s=st*128+j]
            enct = enct_pool.tile([128, N_ST, N_EC, 128], F16, tag="enct",
                                  name=f"enct_{bi}_{sblk}")
            nc.sync.dma_start(
                out=enct.rearrange("p a b j -> p (a b) j"),
                in_=enc16.rearrange("p a e -> p (a e)"),
                transpose=True,
            )
            return enct

        # ---- load pre-transposed weights & small inputs ----
        # All contiguous loads, no device-side weight prep at all: the
        # v4/v5 setups (PE transposes, then W-through-xbar) serialized
        # 30-66us in front of the first energy matmul.
        wt_sb = consts.tile([128, N_DP, NWC, 128], F16)
        nc.scalar.dma_start(out=wt_sb, in_=wt_d)
        hidt_sb = consts.tile([128, N_KC, BP], F16)
        nc.scalar.dma_start(out=hidt_sb, in_=hidt_d)
        bt_sb = consts.tile([128, N_DP], F32)
        nc.scalar.dma_start(out=bt_sb, in_=bt_d)
        vt_sb = consts.tile([128, N_DP], F16)
        nc.scalar.dma_start(out=vt_sb, in_=vt_d)

        # prefetch units 0/1 so their casts lead the DVE queue and their
        # xbars lead the sync ring.
        prefetched = {}
        for pu in (0, 1):
            enc32 = emit_load(0, pu, nc.scalar)
            enc16 = emit_cast(enc32, 0, pu)
            prefetched[pu] = emit_xbar(enc16, 0, pu)

        # preload the exp/tanh activation table early (overlaps with DMAs)
        warm = consts.tile([1, 1], F32)
        nc.vector.memset(warm, 0.0)
        nc.scalar.activation(warm, warm, AF.Tanh)

        # ---- h_projT + bias -> hbT [128, dp, b] ----
        hbt_sb = consts.tile([128, N_DP, BP], F32)
        for dp in range(N_DP):
            ph = ps2.tile([128, 16], F32, tag="sc")
            for kc in range(N_KC):
                nc.tensor.matmul(
                    ph[:, 0:BP],
                    wt_sb[:, dp, kc, :],
                    hidt_sb[:, kc, :],
                    start=(kc == 0), stop=(kc == N_KC - 1),
                )
            nc.vector.tensor_scalar_add(
                hbt_sb[:, dp, :], ph[:, 0:BP], bt_sb[:, dp:dp + 1]
            )

        # ---- main pipeline over 16 (batch, sblk) units ----
        # Per-batch softmax state
        batch_state = {}

        def emit_scores(bi, sblk, energy):
            """Scores matvec + streamed exp with fused partial sums."""
            psc = ps2.tile([1, SBLK], F32, tag="sc")
            for dp in range(N_DP):
                nc.tensor.matmul(
                    psc, vt_sb[:, dp:dp + 1], energy[:, dp, :],
                    start=(dp == 0), stop=(dp == N_DP - 1),
                )
            if sblk == 0:
                prob = small.tile([1, S], F32, tag="prob")
                partials = small.tile([1, N_SBLK], F32, tag="part")
                batch_state[bi] = (prob, partials)
            prob, partials = batch_state[bi]
            if debug_dumps:
                nc.scalar.copy(
                    dbg_scores_sb[:, sblk * SBLK:(sblk + 1) * SBLK], psc)
            # exp without max-subtraction: softmax is shift-invariant and
            # |scores| <~ 40 stays well inside the f32 exp range; accum_out
            # yields the block sum for free (keeps reductions off the DVE).
            nc.scalar.activation(
                prob[:, sblk * SBLK:(sblk + 1) * SBLK], psc, AF.Exp,
                bias=0.0, scale=1.0,
                accum_out=partials[:, sblk:sblk + 1],
            )
            if sblk == N_SBLK - 1:
                ssum = small.tile([1, 1], F32, tag="ssum")
                nc.gpsimd.reduce_sum(ssum, partials, axis=AX.XYZWC)
                rtot = small.tile([1, 1], F32, tag="rtot")
                nc.vector.reciprocal(rtot, ssum)
                attn = small.tile([1, S], F32, tag="attn")
                nc.scalar.activation(attn, prob, AF.Copy, scale=rtot)
                nc.scalar.dma_start(out=out_d[bi], in_=attn)
                if debug_dumps:
                    nc.sync.dma_start(
                        out=dbg_scores[bi:bi + 1, :], in_=dbg_scores_sb)
                del batch_state[bi]

        if debug_dumps:
            dbg_scores_sb = small.tile([1, S], F32, tag="dbgsc")

        prev = None  # pending (bi, sblk, energy) for the scores pipeline
        for bi in range(BP):
            for sblk in range(N_SBLK):
                u = bi * N_SBLK + sblk
                if u in prefetched:
                    enct = prefetched.pop(u)
                else:
                    enc32 = emit_load(bi, sblk, nc.scalar)
                    enc16 = emit_cast(enc32, bi, sblk)
                    enct = emit_xbar(enc16, bi, sblk)
                if debug_dumps and bi == 0 and sblk == 0:
                    nc.sync.dma_start(out=dbg_enct, in_=enct)

                energy = en_pool.tile([128, N_DP, SBLK], F16, tag="energy")
                for dp in range(N_DP):
                    pe = ps.tile([128, SBLK], F32, tag="pe")
                    for ec in range(N_EC):
                        nc.tensor.matmul(
                            pe,
                            wt_sb[:, dp, N_KC + ec, :],
                            enct[:, :, ec, :],
                            start=(ec == 0), stop=(ec == N_EC - 1),
                        )
                    nc.scalar.activation(
                        energy[:, dp, :], pe, AF.Tanh,
                        bias=hbt_sb[:, dp, bi:bi + 1], scale=1.0,
                    )
                    # scores for the previous unit, mid-stream so the PE
                    # never waits on ScalarE's tanh of THIS unit
                    if dp == 1 and prev is not None:
                        emit_scores(*prev)
                        prev = None
                prev = (bi, sblk, energy)

        emit_scores(*prev)

    nc.compile()
    return nc


def _get_nc():
    if "nc" not in _CACHE:
        _CACHE["nc"] = _build()
    return _CACHE["nc"]


def make_in_maps(hidden, encoder_outputs, W, b, v):
    """Host-side sharding + weight layout prep (transpose/cast only)."""
    hidden = np.ascontiguousarray(hidden, dtype=np.float32)
    encoder_outputs = np.ascontiguousarray(encoder_outputs, dtype=np.float32)
    W = np.ascontiguousarray(W, dtype=np.float32)
    b = np.ascontiguousarray(b, dtype=np.float32)
    v = np.ascontiguousarray(v, dtype=np.float32)

    # wt[q, dp, c, j] = W[dp*128+j, c*128+q]
    wt = np.ascontiguousarray(
        W.reshape(N_DP, 128, NWC, 128).transpose(3, 0, 2, 1)
    ).astype(np.float16)
    bt = np.ascontiguousarray(b.reshape(N_DP, 128).T)
    vt = np.ascontiguousarray(v.reshape(N_DP, 128).T).astype(np.float16)

    in_maps = []
    for c in range(N_CORES):
        hid_c = hidden[c * BP:(c + 1) * BP]
        # hidt[p, kc, b] = hidden[b, kc*128+p]
        hidt = np.ascontiguousarray(
            hid_c.reshape(BP, N_KC, 128).transpose(2, 1, 0)
        ).astype(np.float16)
        in_maps.append({
            "hidt": hidt,
            "enc": np.ascontiguousarray(encoder_outputs[c * BP:(c + 1) * BP]),
            "wt": wt,
            "bt": bt,
            "vt": vt,
        })
    return in_maps


def kernel(hidden, encoder_outputs, W, b, v):
    from concourse.bass_utils import run_bass_kernel_spmd

    nc = _get_nc()
    in_maps = make_in_maps(hidden, encoder_outputs, W, b, v)
    r = run_bass_kernel_spmd(nc, in_maps, list(range(N_CORES)))
    out = np.concatenate([r.results[c]["out"] for c in range(N_CORES)], axis=0)
    return out[:, None, :].astype(np.float32)


# revision 20
# speedup vs baseline: 1.1582x; 1.0548x over previous
"""Bahdanau-style attention kernel for Trainium2, data-parallel over batch
across 8 NeuronCores.

Reference computation (per batch b):
    e_proj = enc[b] @ We.T            # [S, D]   (We = W[:, 512:], [D, E])
    energy = tanh(e_proj + hidden[b] @ Wh.T + bias)
    scores = energy @ v               # [S]
    attn   = softmax(scores)          # [1, S]

Shapes: B=32, S=2048, E=1024, D=512.  Each core handles 4 batches.

v2 design (per core), aimed at a continuous PE matmul stream:
  - enc is DMA-loaded f32 (HWDGE, sync ring), cast f32->fp16 on VectorE,
    then transposed fp16 via the DMA xbar (HWDGE transpose on the scalar
    ring) so the contraction dim e lands on partitions.  This keeps ALL
    transposes off the PE: the baseline's PE-transpose phases didn't
    register as PE activity for the HAM clock gate, so the PE oscillated
    between 1.2/2.4 GHz (~104us throttled).  With a pure MM stream the
    PE stays at K=8/8.
  - main matmul: psum[d128, s512] += WeT[e128, d128].T @ encT[e128, s512]
    (fp16 weights -> FWL fast weight load, overlapped via the PE's
    background weight buffer).
  - tanh fused with the (h_proj + b) bias via ScalarE activation
    (per-partition bias, d is the partition dim).
  - scores via TensorE matvec with v, software-pipelined one unit behind
    the energy matmuls so the PE never waits on ScalarE's tanh.
  - softmax without a full max pass: exp bias uses the max of the first
    512-block as a stability proxy (softmax is shift-invariant; f32 exp
    easily covers the residual +-few-unit range), so exp/partial sums
    stream per block and only a tiny normalize tail remains per batch.
"""

import numpy as np

B, S, E, D = 32, 2048, 1024, 512
N_CORES = 8
BP = B // N_CORES  # batches per core = 4
SBLK = 512  # s-block (psum free dim)
N_SBLK = S // SBLK  # 4
N_ST = SBLK // 128  # 4 s-subtiles per block
N_EC = E // 128  # 8 e-chunks
N_DP = D // 128  # 4 d-chunks
N_KC = D // 128  # 4 k-chunks (hidden proj contraction)

_CACHE = {}


NWC = (D + E) // 128  # 12 column-chunks of W


def _build(debug_dumps=False):
    from contextlib import ExitStack

    import concourse.tile as tile
    from concourse import bacc, mybir

    F32 = mybir.dt.float32
    F16 = mybir.dt.float16
    AF = mybir.ActivationFunctionType
    AX = mybir.AxisListType

    nc = bacc.Bacc("TRN2", target_bir_lowering=False, debug=False,
                   num_devices=N_CORES)

    # Weights/small inputs arrive pre-transposed + pre-cast from the host
    # (pure layout transforms; the device does all the FLOPs):
    #   wt[q, dp, c, j]  = W[dp*128+j, c*128+q]   (c<4 -> Wh, c>=4 -> We)
    #   hidt[p, kc, b]   = hidden[b, kc*128+p]
    #   bt[p, dp]        = b[dp*128+p]
    #   vt[p, dp]        = v[dp*128+p]
    wt_d = nc.dram_tensor("wt", [128, N_DP, NWC, 128], F16,
                          kind="ExternalInput").ap()
    hidt_d = nc.dram_tensor("hidt", [128, N_KC, BP], F16,
                            kind="ExternalInput").ap()
    bt_d = nc.dram_tensor("bt", [128, N_DP], F32, kind="ExternalInput").ap()
    vt_d = nc.dram_tensor("vt", [128, N_DP], F16, kind="ExternalInput").ap()
    enc_d = nc.dram_tensor("enc", [BP, S, E], F32, kind="ExternalInput").ap()
    out_d = nc.dram_tensor("out", [BP, S], F32, kind="ExternalOutput").ap()
    if debug_dumps:
        dbg_scores = nc.dram_tensor(
            "dbg_scores", [BP, S], F32, kind="ExternalOutput").ap()
        dbg_enct = nc.dram_tensor(
            "dbg_enct", [128, N_ST, N_EC, 128], F16, kind="ExternalOutput").ap()

    with tile.TileContext(nc) as tc, ExitStack() as ctx:
        consts = ctx.enter_context(tc.tile_pool(name="consts", bufs=1))
        enc_pool = ctx.enter_context(tc.tile_pool(name="enc", bufs=5))
        enc16_pool = ctx.enter_context(tc.tile_pool(name="enc16", bufs=3))
        enct_pool = ctx.enter_context(tc.tile_pool(name="enct", bufs=5))
        en_pool = ctx.enter_context(tc.tile_pool(name="energy", bufs=2))
        small = ctx.enter_context(tc.tile_pool(name="small", bufs=2))
        ps = ctx.enter_context(tc.tile_pool(name="ps", bufs=6, space="PSUM"))
        ps2 = ctx.enter_context(tc.tile_pool(name="ps2", bufs=2, space="PSUM"))

        # ---- per-unit producer helpers ----
        # Engine-role discipline (the point of this structure): the DVE
        # queue carries ONLY the f32->f16 casts.  Any DVE op that
        # transitively depends on PE/ScalarE work would head-of-line-block
        # a later cast (strict FIFO), closing a PE->Scalar->DVE->xbar->PE
        # loop that paces the whole kernel (measured 16us/unit in v3).
        def emit_load(bi, sblk, ring):
            enc32 = enc_pool.tile([128, N_ST, E], F32, tag="enc32",
                                  name=f"enc32_{bi}_{sblk}")
            ring.dma_start(
                out=enc32,
                in_=enc_d[bi, sblk * SBLK:(sblk + 1) * SBLK, :].rearrange(
                    "(st p) e -> p st e", p=128
                ),
            )
            return enc32

        def emit_cast(enc32, bi, sblk):
            enc16 = enc16_pool.tile([128, N_ST, E], F16, tag="enc16",
                                    name=f"enc16_{bi}_{sblk}")
            nc.vector.tensor_copy(enc16[:, 0:2, :], enc32[:, 0:2, :])
            nc.vector.tensor_copy(enc16[:, 2:4, :], enc32[:, 2:4, :])
            return enc16

        def emit_xbar(enc16, bi, sblk):
            # fp16 transpose on the DMA xbar (sync HWDGE ring):
            # out[q, (st, ec), j] = enc16[j, st*1024 + ec*128 + q]
            #   -> enct[q, st, ec, j] = encT[e=ec*128+q, s=st*128+j]
            enct = enct_pool.tile([128, N_ST, N_EC, 128], F16, tag="enct",
                                  name=f"enct_{bi}_{sblk}")
            nc.sync.dma_start(
                out=enct.rearrange("p a b j -> p (a b) j"),
                in_=enc16.rearrange("p a e -> p (a e)"),
                transpose=True,
            )
            return enct

        # ---- load pre-transposed weights & small inputs ----
        # All contiguous loads, no device-side weight prep at all: the
        # v4/v5 setups (PE transposes, then W-through-xbar) serialized
        # 30-66us in front of the first energy matmul.  Setup loads ride
        # the sync ring (idle until the first xbar); enc loads ride the
        # scalar ring.
        wt_sb = consts.tile([128, N_DP, NWC, 128], F16)
        nc.sync.dma_start(out=wt_sb, in_=wt_d)
        hidt_sb = consts.tile([128, N_KC, BP], F16)
        nc.sync.dma_start(out=hidt_sb, in_=hidt_d)
        bt_sb = consts.tile([128, N_DP], F32)
        nc.sync.dma_start(out=bt_sb, in_=bt_d)
        vt_sb = consts.tile([128, N_DP], F16)
        nc.sync.dma_start(out=vt_sb, in_=vt_d)

        # prefetch: loads for units 0-2, cast+xbar for units 0/1, so the
        # casts lead the DVE queue and the xbars lead the sync ring.
        LOOKAHEAD = 3  # units of load-emission skew (see main loop)
        pending_loads = {u: emit_load(u // N_SBLK, u % N_SBLK, nc.scalar)
                         for u in range(LOOKAHEAD)}
        prefetched = {}
        for pu in (0, 1):
            enc16 = emit_cast(pending_loads.pop(pu), 0, pu)
            prefetched[pu] = emit_xbar(enc16, 0, pu)

        # preload the exp/tanh activation table early (overlaps with DMAs)
        warm = consts.tile([1, 1], F32)
        nc.vector.memset(warm, 0.0)
        nc.scalar.activation(warm, warm, AF.Tanh)

        # ---- h_projT + bias -> hbT [128, dp, b] ----
        hbt_sb = consts.tile([128, N_DP, BP], F32)
        for dp in range(N_DP):
            ph = ps2.tile([128, 16], F32, tag="sc")
            for kc in range(N_KC):
                nc.tensor.matmul(
                    ph[:, 0:BP],
                    wt_sb[:, dp, kc, :],
                    hidt_sb[:, kc, :],
                    start=(kc == 0), stop=(kc == N_KC - 1),
                )
            nc.vector.tensor_scalar_add(
                hbt_sb[:, dp, :], ph[:, 0:BP], bt_sb[:, dp:dp + 1]
            )

        # ---- main pipeline over 16 (batch, sblk) units ----
        # Per-batch softmax state
        batch_state = {}

        def emit_scores(bi, sblk, energy):
            """Scores matvec + streamed exp with fused partial sums."""
            psc = ps2.tile([1, SBLK], F32, tag="sc")
            for dp in range(N_DP):
                nc.tensor.matmul(
                    psc, vt_sb[:, dp:dp + 1], energy[:, dp, :],
                    start=(dp == 0), stop=(dp == N_DP - 1),
                )
            if sblk == 0:
                prob = small.tile([1, S], F32, tag="prob")
                partials = small.tile([1, N_SBLK], F32, tag="part")
                batch_state[bi] = (prob, partials)
            prob, partials = batch_state[bi]
            if debug_dumps:
                nc.scalar.copy(
                    dbg_scores_sb[:, sblk * SBLK:(sblk + 1) * SBLK], psc)
            # exp without max-subtraction: softmax is shift-invariant and
            # |scores| <~ 40 stays well inside the f32 exp range; accum_out
            # yields the block sum for free (keeps reductions off the DVE).
            nc.scalar.activation(
                prob[:, sblk * SBLK:(sblk + 1) * SBLK], psc, AF.Exp,
                bias=0.0, scale=1.0,
                accum_out=partials[:, sblk:sblk + 1],
            )
            if sblk == N_SBLK - 1:
                ssum = small.tile([1, 1], F32, tag="ssum")
                nc.gpsimd.reduce_sum(ssum, partials, axis=AX.XYZWC)
                rtot = small.tile([1, 1], F32, tag="rtot")
                nc.vector.reciprocal(rtot, ssum)
                attn = small.tile([1, S], F32, tag="attn")
                nc.scalar.activation(attn, prob, AF.Copy, scale=rtot)
                nc.scalar.dma_start(out=out_d[bi], in_=attn)
                if debug_dumps:
                    nc.sync.dma_start(
                        out=dbg_scores[bi:bi + 1, :], in_=dbg_scores_sb)
                del batch_state[bi]

        if debug_dumps:
            dbg_scores_sb = small.tile([1, S], F32, tag="dbgsc")

        N_UNITS = BP * N_SBLK
        prev = None  # pending (bi, sblk, energy) for the scores pipeline
        for bi in range(BP):
            for sblk in range(N_SBLK):
                u = bi * N_SBLK + sblk
                # emit the load for unit u+LOOKAHEAD now, BEFORE this
                # unit's tanh ops join the scalar FIFO — a load dispatch
                # queued after a tanh inherits the tanh's PE dependency
                # (head-of-line), which measured 15-20us/unit of lag.
                un = u + LOOKAHEAD
                if un < N_UNITS:
                    pending_loads[un] = emit_load(
                        un // N_SBLK, un % N_SBLK, nc.scalar)
                if u in prefetched:
                    enct = prefetched.pop(u)
                else:
                    enc16 = emit_cast(pending_loads.pop(u), bi, sblk)
                    enct = emit_xbar(enc16, bi, sblk)
                if debug_dumps and bi == 0 and sblk == 0:
                    nc.sync.dma_start(out=dbg_enct, in_=enct)

                energy = en_pool.tile([128, N_DP, SBLK], F16, tag="energy")
                for dp in range(N_DP):
                    pe = ps.tile([128, SBLK], F32, tag="pe")
                    for ec in range(N_EC):
                        nc.tensor.matmul(
                            pe,
                            wt_sb[:, dp, N_KC + ec, :],
                            enct[:, :, ec, :],
                            start=(ec == 0), stop=(ec == N_EC - 1),
                        )
                    nc.scalar.activation(
                        energy[:, dp, :], pe, AF.Tanh,
                        bias=hbt_sb[:, dp, bi:bi + 1], scale=1.0,
                    )
                    # scores for the previous unit, mid-stream so the PE
                    # never waits on ScalarE's tanh of THIS unit
                    if dp == 1 and prev is not None:
                        emit_scores(*prev)
                        prev = None
                prev = (bi, sblk, energy)

        emit_scores(*prev)

    nc.compile()
    return nc


def _get_nc():
    if "nc" not in _CACHE:
        _CACHE["nc"] = _build()
    return _CACHE["nc"]


def make_in_maps(hidden, encoder_outputs, W, b, v):
    """Host-side sharding + weight layout prep (transpose/cast only)."""
    hidden = np.ascontiguousarray(hidden, dtype=np.float32)
    encoder_outputs = np.ascontiguousarray(encoder_outputs, dtype=np.float32)
    W = np.ascontiguousarray(W, dtype=np.float32)
    b = np.ascontiguousarray(b, dtype=np.float32)
    v = np.ascontiguousarray(v, dtype=np.float32)

    # wt[q, dp, c, j] = W[dp*128+j, c*128+q]
    wt = np.ascontiguousarray(
        W.reshape(N_DP, 128, NWC, 128).transpose(3, 0, 2, 1)
    ).astype(np.float16)
    bt = np.ascontiguousarray(b.reshape(N_DP, 128).T)
    vt = np.ascontiguousarray(v.reshape(N_DP, 128).T).astype(np.float16)

    in_maps = []
    for c in range(N_CORES):
        hid_c = hidden[c * BP:(c + 1) * BP]
        # hidt[p, kc, b] = hidden[b, kc*128+p]
        hidt = np.ascontiguousarray(
            hid_c.reshape(BP, N_KC, 128).transpose(2, 1, 0)
        ).astype(np.float16)
        in_maps.append({
            "hidt": hidt,
            "enc": np.ascontiguousarray(encoder_outputs[c * BP:(c + 1) * BP]),
            "wt": wt,
            "bt": bt,
            "vt": vt,
        })
    return in_maps


def kernel(hidden, encoder_outputs, W, b, v):
    from concourse.bass_utils import run_bass_kernel_spmd

    nc = _get_nc()
    in_maps = make_in_maps(hidden, encoder_outputs, W, b, v)
    r = run_bass_kernel_spmd(nc, in_maps, list(range(N_CORES)))
    out = np.concatenate([r.results[c]["out"] for c in range(N_CORES)], axis=0)
    return out[:, None, :].astype(np.float32)
